# revision 14
# baseline (speedup 1.0000x reference)
"""BitNet 3-layer MLP (B=131072, D=256) on 8 TRN2 NeuronCores, data-parallel.

Per-core shard: 16384 rows. Math f32-exact relative to the reference up to
benign summation-order differences:

  per layer:  LayerNorm(row) -> global-absmax int8 fake-quant -> (+-1 W) matmul
              -> scale (-> relu for layers 1,2)

v2 restructure (vs v1 two-sweep): single fused sweep per layer.
  - Stats (BNStats mean/var + row max) for layer L+1 are computed inside
    layer L's sweep right after each group's epilogue writes the arena, so
    the DVE stats work overlaps the PE/Scalar/GpSimd/DMA work of the same
    sweep instead of forming a serial DVE-only phase.
  - Quantized activations are fp16 with a +1536 offset (fp addition rounds
    to integer, round-half-even); the offset is cancelled INSIDE the psum
    accumulation by a K=1 correction matmul (lhsT = const -1536 row,
    rhs = colsum(wq) repeated), so no DVE subtract pass is needed.
  - Row max uses a single 2-byte-packed tensor_reduce (2x DVE mode) on the
    int16 arena instead of tensor_tensor max trees.
  - The quantize tensor_scalar is split between Scalar/GpSimd/DVE engines.
  - Transposes are group-sized (2KB/partition) to amortize HWDGE overhead.
  - activations between layers are exact integers (relu of +-1-weight matmul
    of int8 values) stored as int16 in SBUF; max |pre-act| ~2.3k.
  - layer scaling beta*gamma/127 cancels in the next LayerNorm, so it is
    only applied in the final layer.
"""
import numpy as np
from contextlib import ExitStack

from concourse import bass, tile, mybir
from concourse import bacc
from concourse.bass_utils import run_bass_kernel_spmd
from concourse import bass_isa

P = 128
D = 256
NCORES = 8
B = 131072
B_LOC = B // NCORES          # 16384
T = B_LOC // P               # 128 tiles
G = 8                        # tiles per group
NGRP = T // G                # 16 groups
OFF = 1536.0                 # fp16 rounding offset
LN_EPS = 1e-5
QB = 127.0

f32 = mybir.dt.float32
f16 = mybir.dt.float16
i16 = mybir.dt.int16
Alu = mybir.AluOpType
Act = mybir.ActivationFunctionType

# quantize tile assignment per group of 8 tiles: (n_dve, n_scalar, rest=gpsimd)
Q_SPLIT = {0: (0, 2), 1: (0, 2), 2: (5, 3)}
# prologue: the FIRST SC_BN_GROUPS groups (loaded earliest) do mean/var on
# the scalar engine via Square/Identity accumulate; the rest on DVE BNStats.
SC_BN_GROUPS = 6
# number of x groups kept resident in SBUF f32 (skips the layer-0 reload)
XK = 5


def build_nc():
    nc = bacc.Bacc("TRN2", target_bir_lowering=False, debug=False,
                   num_devices=NCORES)

    x_d = nc.dram_tensor("x", [B_LOC, D], f32, kind="ExternalInput")
    w_d = [nc.dram_tensor(f"W{i+1}", [D, D], f32, kind="ExternalInput")
           for i in range(3)]
    out_d = nc.dram_tensor("out", [B_LOC, D], f32, kind="ExternalOutput")

    with tile.TileContext(nc) as tc:
        with ExitStack() as ctx:
            wt = ctx.enter_context(tc.tile_pool(name="wt", bufs=1))
            stats = ctx.enter_context(tc.tile_pool(name="stats", bufs=1))
            xr1 = ctx.enter_context(tc.tile_pool(name="xr1", bufs=2))
            xr2 = ctx.enter_context(tc.tile_pool(name="xr2", bufs=3))
            u16p = ctx.enter_context(tc.tile_pool(name="u16p", bufs=3))
            u16tp = ctx.enter_context(tc.tile_pool(name="u16tp", bufs=3))
            treep = ctx.enter_context(tc.tile_pool(name="treep", bufs=2))
            smallp = ctx.enter_context(tc.tile_pool(name="smallp", bufs=2))
            psum = ctx.enter_context(tc.tile_pool(name="psum", bufs=4, space="PSUM"))
            dram = ctx.enter_context(tc.tile_pool(name="dram", bufs=2, space="DRAM"))

            arena = wt.tile([P, NGRP, G, D], i16)
            xkeep = wt.tile([P, XK, G, D], f32)

            # ---------------- constants ----------------
            onesf = wt.tile([P, 1], f32)
            nc.vector.memset(onesf[:], 1.0)
            ones16 = wt.tile([P, 1], f16)
            nc.vector.memset(ones16[:], 1.0)
            repl = wt.tile([1, P], f32)          # K=1 replicate row
            nc.vector.memset(repl[:], 1.0)
            negoff = wt.tile([1, P], f16)        # K=1 row of -OFF
            nc.vector.memset(negoff[:], -OFF)
            epst = wt.tile([P, 1], f32)
            nc.vector.memset(epst[:], LN_EPS)

            # ---------------- weight prep ----------------
            wqT = []     # [128, 2, 256] fp16: wqT[k_in_band, band, j]
            colsum = []  # [1, 256] f16: colsum_j = sum_k wq[k, j]
            beta = []    # [1, 1] f32
            for li in range(3):
                wf = wt.tile([P, 2, D], f32, tag="wf")
                nc.sync.dma_start(out=wf[:], in_=w_d[li][:].rearrange(
                    "(a p) d -> p a d", p=P))
                # alpha = mean(W)
                rs = wt.tile([P, 2], f32, tag="rs")
                nc.vector.tensor_reduce(out=rs[:], in_=wf[:],
                                        axis=mybir.AxisListType.X, op=Alu.add)
                rv = wt.tile([P, 1], f32, tag="rv")
                nc.vector.tensor_tensor(out=rv[:], in0=rs[:, 0:1], in1=rs[:, 1:2],
                                        op=Alu.add)
                aps = psum.tile([2, P], f32, tag="mm_ps")
                nc.tensor.matmul(aps[0:1, 0:1], lhsT=onesf[:], rhs=rv[:],
                                 start=True, stop=True)
                alpha = wt.tile([1, 1], f32, tag="alpha")
                nc.scalar.activation(out=alpha[:], in_=aps[0:1, 0:1], func=Act.Copy,
                                     scale=1.0 / (D * D))
                abc_ps = psum.tile([P, 1], f32, tag="mm_ps")
                nc.tensor.matmul(abc_ps[:], lhsT=repl[:], rhs=alpha[:],
                                 start=True, stop=True)
                abc = wt.tile([P, 1], f32, tag="abc")
                nc.vector.tensor_copy(out=abc[:], in_=abc_ps[:])
                wc = wt.tile([P, 2, D], f32, tag="wc")
                nc.vector.tensor_scalar(out=wc[:], in0=wf[:], scalar1=abc[:],
                                        scalar2=None, op0=Alu.subtract)
                # beta = mean|wc|
                ba = wt.tile([P, 2], f32, tag="ba")
                nc.vector.tensor_reduce(out=ba[:], in_=wc[:],
                                        axis=mybir.AxisListType.X, op=Alu.add,
                                        apply_absolute_value=True)
                bv = wt.tile([P, 1], f32, tag="bv")
                nc.vector.tensor_tensor(out=bv[:], in0=ba[:, 0:1], in1=ba[:, 1:2],
                                        op=Alu.add)
                bps = psum.tile([2, P], f32, tag="mm_ps")
                nc.tensor.matmul(bps[0:1, 0:1], lhsT=onesf[:], rhs=bv[:],
                                 start=True, stop=True)
                bt = wt.tile([1, 1], f32, tag=f"beta{li}")
                nc.scalar.activation(out=bt[:], in_=bps[0:1, 0:1], func=Act.Copy,
                                     scale=1.0 / (D * D))
                beta.append(bt)
                # wq = sign(wc) in fp16
                wq16 = wt.tile([P, 2, D], f16, tag="wq16")
                nc.vector.tensor_scalar(out=wq16[:], in0=wc[:], scalar1=0.0,
                                        scalar2=2.0, op0=Alu.is_gt, op1=Alu.mult)
                nc.vector.tensor_scalar(out=wq16[:], in0=wq16[:], scalar1=1.0,
                                        scalar2=None, op0=Alu.subtract)
                wqt = wt.tile([P, 2, D], f16, tag=f"wqT{li}")
                for a in range(2):
                    for k in range(2):
                        nc.sync.dma_start_transpose(
                            out=wqt[:, k, a * P:(a + 1) * P],
                            in_=wq16[:, a, k * P:(k + 1) * P])
                wqT.append(wqt)
                # colsum_j = sum_k wq[k, j] via ones-matmul over both k halves
                csps = psum.tile([1, D], f32, tag="mm_ps")
                nc.tensor.matmul(csps[:], lhsT=ones16[:], rhs=wqt[:, 0, :],
                                 start=True, stop=False)
                nc.tensor.matmul(csps[:], lhsT=ones16[:], rhs=wqt[:, 1, :],
                                 start=False, stop=True)
                cs = wt.tile([1, 2, D], f16, tag=f"colsum{li}")
                nc.vector.tensor_copy(out=cs[:, 0, :], in_=csps[:])
                nc.vector.tensor_copy(out=cs[:, 1, :], in_=csps[:])
                colsum.append(cs)

            # stats arrays (per current layer input)
            bnt = stats.tile([P, T // 2, 6], f32, tag="bnt")
            rmxf = stats.tile([P, T], f32, tag="rmxf")     # layer 0 (f32 x)
            rmnf = stats.tile([P, T], f32, tag="rmnf")
            rmxi = stats.tile([P, T], i16, tag="rmxi")     # layers 1,2 (i16)
            sx = stats.tile([P, T], f32, tag="sx")         # scalar-accum sums
            sxx = stats.tile([P, T], f32, tag="sxx")
            dumf = wt.tile([P, D], f32, tag="dumf")        # scratch act output

            def bn_group(grp, g):
                """BNStats mean/var for 8 tiles of one group (DVE)."""
                for i in range(0, G, 2):
                    _in3d = grp[:, i:i + 2, :].rearrange("p t d -> p d t")
                    nc.vector.add_instruction(mybir.InstBNStats(
                        name=nc.get_next_instruction_name(),
                        ins=[nc.vector.lower_ap(_in3d)],
                        outs=[nc.vector.lower_ap(
                            bnt[:, (g * G + i) // 2, :])]))

            def bn_group_scalar(grp, g):
                """sum/sumsq for 8 tiles via scalar-engine accumulate."""
                for i in range(G):
                    t = g * G + i
                    nc.scalar.activation(
                        out=dumf[:], in_=grp[:, i, :], func=Act.Identity,
                        accum_out=sx[:, t:t + 1])
                    nc.scalar.activation(
                        out=dumf[:], in_=grp[:, i, :], func=Act.Square,
                        accum_out=sxx[:, t:t + 1])

            # ---------------- layer-0 stats prologue ----------------
            # loads issued from the gpsimd (SWDGE) queue so the Sync HWDGE
            # ring stays free; first XK groups stay resident in SBUF.
            for g in range(NGRP):
                ldeng = nc.gpsimd if g % 2 == 0 else nc.sync
                if g < XK:
                    xg = xkeep[:, g, :, :]
                    ldeng.dma_start(out=xg, in_=x_d[
                        g * G * P:(g + 1) * G * P, :].rearrange(
                        "(t p) d -> p t d", p=P))
                else:
                    xg = xr1.tile([P, G, D], f32, tag="xr1")
                    ldeng.dma_start(out=xg[:], in_=x_d[
                        g * G * P:(g + 1) * G * P, :].rearrange(
                        "(t p) d -> p t d", p=P))
                    xg = xg[:]
                if g < SC_BN_GROUPS:
                    bn_group_scalar(xg, g)
                else:
                    bn_group(xg, g)
                nc.vector.tensor_reduce(
                    out=rmxf[:, g * G:(g + 1) * G], in_=xg,
                    axis=mybir.AxisListType.X, op=Alu.max)
                nc.vector.tensor_reduce(
                    out=rmnf[:, g * G:(g + 1) * G], in_=xg,
                    axis=mybir.AxisListType.X, op=Alu.min)

            # ---------------- per-layer scale math + fused sweep ----------------
            for li in range(3):
                last = li == 2

                # ---- per-row stat math on [P, T] ----
                # columns [0, ks) (layer 0 only) come from scalar-engine
                # sum/sumsq accumulate; columns [ks, T) from DVE BNStats.
                ks = SC_BN_GROUPS * G if li == 0 else 0
                mu = stats.tile([P, T], f32, tag="mu")
                var = stats.tile([P, T], f32, tag="var")
                nc.vector.tensor_copy(out=mu[:, ks:T].rearrange(
                    "p (t two) -> p t two", two=2)[:, :, 0],
                    in_=bnt[:, ks // 2:T // 2, 1])
                nc.vector.tensor_copy(out=mu[:, ks:T].rearrange(
                    "p (t two) -> p t two", two=2)[:, :, 1],
                    in_=bnt[:, ks // 2:T // 2, 4])
                nc.vector.tensor_scalar(out=var[:, ks:T].rearrange(
                    "p (t two) -> p t two", two=2)[:, :, 0],
                    in0=bnt[:, ks // 2:T // 2, 2],
                    scalar1=1.0 / D, scalar2=None, op0=Alu.mult)
                nc.vector.tensor_scalar(out=var[:, ks:T].rearrange(
                    "p (t two) -> p t two", two=2)[:, :, 1],
                    in0=bnt[:, ks // 2:T // 2, 5],
                    scalar1=1.0 / D, scalar2=None, op0=Alu.mult)
                if ks > 0:
                    # mu = sx/D ; var = sxx/D - mu^2
                    nc.vector.tensor_scalar(
                        out=mu[:, 0:ks], in0=sx[:, 0:ks],
                        scalar1=1.0 / D, scalar2=None, op0=Alu.mult)
                    musq = stats.tile([P, T], f32, tag="musq")
                    nc.vector.tensor_tensor(
                        out=musq[:, 0:ks], in0=mu[:, 0:ks], in1=mu[:, 0:ks],
                        op=Alu.mult)
                    nc.vector.tensor_scalar(
                        out=var[:, 0:ks], in0=sxx[:, 0:ks],
                        scalar1=1.0 / D, scalar2=None, op0=Alu.mult)
                    nc.vector.tensor_tensor(
                        out=var[:, 0:ks], in0=var[:, 0:ks], in1=musq[:, 0:ks],
                        op=Alu.subtract)
                rstd = stats.tile([P, T], f32, tag="rstd")
                nc.scalar.activation(out=rstd[:], in_=var[:], func=Act.Sqrt,
                                     bias=epst[:], scale=1.0)
                nc.vector.reciprocal(out=rstd[:], in_=rstd[:])
                # centered absmax * rstd
                a1 = stats.tile([P, T], f32, tag="a1")
                if li == 0:
                    nc.vector.tensor_tensor(out=a1[:], in0=rmxf[:], in1=mu[:],
                                            op=Alu.subtract)
                    a2 = stats.tile([P, T], f32, tag="a2")
                    nc.vector.tensor_tensor(out=a2[:], in0=mu[:], in1=rmnf[:],
                                            op=Alu.subtract)
                    nc.vector.tensor_tensor(out=a1[:], in0=a1[:], in1=a2[:],
                                            op=Alu.max)
                else:
                    # inputs are relu'd (>=0, row min is 0): A = max(rmx-mu, mu)
                    nc.vector.tensor_tensor(out=a1[:], in0=rmxi[:], in1=mu[:],
                                            op=Alu.subtract)
                    nc.vector.tensor_tensor(out=a1[:], in0=a1[:], in1=mu[:],
                                            op=Alu.max)
                nc.vector.tensor_tensor(out=a1[:], in0=a1[:], in1=rstd[:],
                                        op=Alu.mult)
                gl = stats.tile([P, 1], f32, tag="gl")
                nc.vector.tensor_reduce(out=gl[:], in_=a1[:],
                                        axis=mybir.AxisListType.X, op=Alu.max)
                nc.gpsimd.partition_all_reduce(gl[:], gl[:], channels=P,
                                               reduce_op=bass_isa.ReduceOp.max)
                nc.vector.tensor_scalar(out=gl[0:1, :], in0=gl[0:1, :],
                                        scalar1=1e-8, scalar2=None, op0=Alu.max)

                # ---- AllGather of local gamma candidate ----
                snd_sb = smallp.tile([1, 8], f32, tag="snd_sb")
                nc.gpsimd.tensor_copy(out=snd_sb[:],
                                      in_=gl[0:1, 0:1].broadcast_to((1, 8)))
                snd = dram.tile([1, 8], f32, tag="snd")
                rcv = dram.tile([1, 8 * NCORES], f32, tag="rcv")
                nc.gpsimd.dma_start(out=snd[:], in_=snd_sb[:])
                nc.gpsimd.collective_compute(
                    "AllGather", Alu.bypass, ins=[snd[:].opt()],
                    outs=[rcv[:].opt()],
                    replica_groups=[list(range(NCORES))])
                g64 = smallp.tile([1, 8 * NCORES], f32, tag="g64")
                nc.gpsimd.dma_start(out=g64[:], in_=rcv[:])
                gam = smallp.tile([1, 1], f32, tag="gam")
                nc.vector.tensor_reduce(out=gam[:], in_=g64[:],
                                        axis=mybir.AxisListType.X, op=Alu.max)
                gi = smallp.tile([1, 1], f32, tag="gi")
                nc.vector.reciprocal(out=gi[:], in_=gam[:])
                nc.vector.tensor_scalar(out=gi[:], in0=gi[:], scalar1=QB,
                                        scalar2=None, op0=Alu.mult)
                gbc = smallp.tile([P, 1], f32, tag="gbc")
                nc.gpsimd.partition_broadcast(gbc[:], gi[:])
                s1 = stats.tile([P, T], f32, tag="s1")
                nc.vector.tensor_scalar(out=s1[:], in0=rstd[:], scalar1=gbc[:],
                                        scalar2=None, op0=Alu.mult)
                tp = stats.tile([P, T], f32, tag="tp")
                nc.vector.tensor_tensor(out=tp[:], in0=mu[:], in1=s1[:],
                                        op=Alu.mult)
                nc.vector.tensor_scalar(out=tp[:], in0=tp[:], scalar1=-1.0,
                                        scalar2=OFF, op0=Alu.mult, op1=Alu.add)

                if last:
                    # c3 = beta * gamma / 127, broadcast to [P, 1]
                    c3 = smallp.tile([1, 1], f32, tag="c3")
                    nc.vector.tensor_tensor(out=c3[:], in0=beta[li][:],
                                            in1=gam[:], op=Alu.mult)
                    nc.vector.tensor_scalar(out=c3[:], in0=c3[:], scalar1=1.0 / QB,
                                            scalar2=None, op0=Alu.mult)
                    c3bc = smallp.tile([P, 1], f32, tag="c3bc")
                    nc.gpsimd.partition_broadcast(c3bc[:], c3[:])

                # ---- fused sweep, software-pipelined with stage lag:
                #      stage1(g): quantize + transpose
                #      stage2(g-1): matmuls
                #      stage3(g-2): epilogue + next-layer stats
                # The lag keeps every engine FIFO's head dependency satisfied
                # (no MM(g) -> epilogue(g) -> quantize(g+1) serialization loop
                # through the scalar engine queue).
                n_dve, n_sc = Q_SPLIT[li]
                hTs = {}
                pss = {}

                xg2s = {}

                def prefetch(g):
                    # issue the layer-0 reload a couple of groups early so
                    # quantize never waits on the DMA
                    xg2 = xr2.tile([P, G, D], f32, tag="xr2")
                    nc.scalar.dma_start(out=xg2[:], in_=x_d[
                        g * G * P:(g + 1) * G * P, :].rearrange(
                        "(t p) d -> p t d", p=P))
                    xg2s[g] = xg2

                def stage1(g, li=li, n_dve=n_dve, n_sc=n_sc, s1=s1, tp=tp):
                    if li == 0:
                        src_g = xkeep[:, g, :, :] if g < XK \
                            else xg2s.pop(g)[:]
                    else:
                        src_g = arena[:, g, :, :]
                    st = u16p.tile([P, G, D], f16, tag="u16")
                    for i in range(G):
                        t = g * G + i
                        if i < n_dve:
                            eng = nc.vector
                        elif i < n_dve + n_sc:
                            eng = nc.scalar
                        else:
                            eng = nc.gpsimd
                        if eng is nc.scalar:
                            # values are always in [1409,1663] so Relu == id
                            nc.scalar.activation(
                                out=st[:, i, :], in_=src_g[:, i, :],
                                func=Act.Relu, bias=tp[:, t:t + 1],
                                scale=s1[:, t:t + 1])
                        else:
                            eng.tensor_scalar(
                                out=st[:, i, :], in0=src_g[:, i, :],
                                scalar1=s1[:, t:t + 1], scalar2=tp[:, t:t + 1],
                                op0=Alu.mult, op1=Alu.add)
                    hT = u16tp.tile([P, 2 * G, P], f16, tag="u16T")
                    nc.sync.dma_start_transpose(
                        out=hT[:], in_=st[:].rearrange("p a d -> p (a d)"))
                    hTs[g] = hT

                def stage2(g, li=li):
                    hT = hTs.pop(g)
                    ps = psum.tile([P, G // 2, D], f32, tag="mm_ps")
                    ps2 = psum.tile([P, G // 2, D], f32, tag="mm_ps")
                    for h, pst in enumerate((ps, ps2)):
                        # K=1 correction first (clears psum): -OFF*colsum(wq)
                        for b in range(2):
                            nc.tensor.matmul(
                                pst[:, 2 * b:2 * b + 2, :], lhsT=negoff[:],
                                rhs=colsum[li][:], start=True, stop=False,
                                skip_group_check=True)
                        for i in range(4):
                            sl = pst[:, i, :]
                            ti = h * 4 + i
                            nc.tensor.matmul(sl, lhsT=hT[:, 2 * ti, :],
                                             rhs=wqT[li][:, 0, :],
                                             start=False, stop=False,
                                             skip_group_check=True)
                            nc.tensor.matmul(sl, lhsT=hT[:, 2 * ti + 1, :],
                                             rhs=wqT[li][:, 1, :],
                                             start=False, stop=True,
                                             skip_group_check=True)
                    pss[g] = (ps, ps2)

                def stage3(g, li=li, last=last, c3bc=c3bc if last else None):
                    ps, ps2 = pss.pop(g)
                    if not last:
                        nc.scalar.activation(
                            out=arena[:, g, 0:4, :],
                            in_=ps[:], func=Act.Relu, scale=1.0)
                        nc.scalar.activation(
                            out=arena[:, g, 4:8, :],
                            in_=ps2[:], func=Act.Relu, scale=1.0)
                        # fused stats for the next layer's input
                        grp = arena[:, g, :, :]
                        bn_group(grp, g)
                        # row max via 2x-packed TT folds 256->128->64->32
                        tr = treep.tile([P, G, P], i16, tag="tri")
                        nc.vector.tensor_tensor(
                            out=tr[:], in0=grp[:, :, 0:P], in1=grp[:, :, P:D],
                            op=Alu.max)
                        nc.vector.tensor_tensor(
                            out=tr[:, :, 0:P // 2], in0=tr[:, :, 0:P // 2],
                            in1=tr[:, :, P // 2:P], op=Alu.max)
                        nc.vector.tensor_tensor(
                            out=tr[:, :, 0:P // 4], in0=tr[:, :, 0:P // 4],
                            in1=tr[:, :, P // 4:P // 2], op=Alu.max)
                        nc.vector.tensor_reduce(
                            out=rmxi[:, g * G:(g + 1) * G],
                            in_=tr[:, :, 0:P // 4],
                            axis=mybir.AxisListType.X, op=Alu.max)
                    else:
                        # stage the scaled output through a spare xkeep slot
                        # (x is consumed by now): 5-deep buffering detaches
                        # the store DMA latency from the psum recycle path.
                        og = xkeep[:, g % XK, :, :].bitcast(f32)
                        nc.vector.tensor_scalar(
                            out=og[:, 0:4, :], in0=ps[:],
                            scalar1=c3bc[:], scalar2=None, op0=Alu.mult)
                        nc.scalar.activation(
                            out=og[:, 4:8, :], in_=ps2[:],
                            func=Act.Copy, scale=c3bc[:])
                        nc.gpsimd.dma_start(
                            out=out_d[g * G * P:(g + 1) * G * P, :].rearrange(
                                "(t p) d -> p t d", p=P), in_=og)

                for g in range(NGRP + 2):
                    if li == 0 and XK <= g + 2 < NGRP:
                        prefetch(g + 2)
                    if g < NGRP:
                        stage1(g)
                    if 1 <= g <= NGRP:
                        stage2(g - 1)
                    if g >= 2:
                        stage3(g - 2)

    nc.compile()
    return nc


_NC_CACHE = None


def _get_nc():
    global _NC_CACHE
    if _NC_CACHE is None:
        _NC_CACHE = build_nc()
    return _NC_CACHE


def run(inputs, trace=False, **kw):
    nc = _get_nc()
    x = inputs["x"]
    in_maps = []
    for c in range(NCORES):
        in_maps.append({
            "x": np.ascontiguousarray(x[c * B_LOC:(c + 1) * B_LOC]),
            "W1": inputs["W1"], "W2": inputs["W2"], "W3": inputs["W3"],
        })
    res = run_bass_kernel_spmd(nc, in_maps, core_ids=list(range(NCORES)),
                               trace=trace, **kw)
    out = np.concatenate([r["out"] for r in res.results], axis=0)
    return out, res


def kernel(**inputs):
    out, _ = run(inputs)
    return out
